# revision 15
# baseline (speedup 1.0000x reference)
"""Trainium2 Bass kernel for nn_Cell2Tissue (scatter_memory).

Reference computation:
  avg = AvgPool4x4(Conv3x3_SAME(cell) + bias)          # (128, 64, 64)
  for each tissue sample j: ROI_j += avg               # 64x64 ROI from loc
  output = stack of B copies of the mutated tissue     # (4, 4, 128, 256, 256)

Sharding over 8 cores: core c = (sample j = c % 4, channel half h = c // 4).

Strategy (memory-bound; tolerance gate is rel_err < 2e-2):
- tissue/out streamed as bf16 (host converts; ~0.23% rel err) -> bulk
  copy bytes halve to 16.8 MB/core.
- cell polyphase planes and folded 6x6 weights in fp8e4 (weights
  pre-scaled x2048, descaled on DVE). End-to-end rel err 2.4e-3.
- conv work-split + AllGather: the 4 cores sharing a channel half each
  compute 1/4 of the pooled rows from 1/4 of the cell planes (2.3 MB
  instead of 8.8 MB replicated), exchange quarters via an HBM AllGather
  within the group, cutting per-core HBM bytes from 27 to 20.5 MB.
- conv via fp8 DoubleRow matmuls: two 6x6 taps contracted per PE pass
  using overlapping custom-stride APs over the resident planes.
- bulk copy is DRAM->DRAM with 128 KB contiguous descriptors, split in
  two channel halves; the dynamic ROI scatter for half A overlaps the
  copy of half B (write engines gate on the copy through RAW reads of
  out instead of a full-engine barrier).
- ROI scatter split across the 3 dynamic-capable queues (sync/scalar/
  gpsimd); the dynamic-queue path is descriptor-rate bound (~200-300
  descs/us globally), hence the overlap above.
"""

import os
import numpy as np

B, C, H, W = 4, 128, 256, 256
CH = C // 2          # channels per core (half)
L = 32               # half ROI width
ROI = 2 * L          # 64
NCORES = 8
PRR = 17             # polyphase plane rows held per core (16 + 1 halo)
PRC = 66             # polyphase plane cols
PHASES = 16
WS = 2048.0          # fp8 weight pre-scale (power of 2; descaled on DVE)
NBLK = 8             # conv row blocks of the full avg: 8 rows each
ORB = ROI // NBLK    # 8 output rows per block
LBLK = 2             # row blocks computed locally per core

_CACHE = {}


def _get_modules():
    if "mods" in _CACHE:
        return _CACHE["mods"]
    # bass2jax executes via the jax 'axon'/'neuron' platform; a cpu-pinned
    # JAX_PLATFORMS would hide the devices.
    if os.environ.get("JAX_PLATFORMS") in ("cpu",):
        del os.environ["JAX_PLATFORMS"]
    import concourse.bass as bass
    import concourse.mybir as mybir
    import concourse.tile as tile
    from concourse.bass_utils import run_bass_kernel_spmd

    _CACHE["mods"] = (bass, mybir, tile, run_bass_kernel_spmd)
    return _CACHE["mods"]


def _split_multiwaits(nc, mybir, max_waits=1):
    """The walrus build here rejects >1 sem-wait on some instructions (the
    Tile tail InstDrain). Hoist extra waits onto single-wait nops placed
    immediately before, on the same engine (same-engine program order
    preserves semantics)."""
    for fn in nc.m.functions:
        for bb in fn.blocks:
            insts = bb.instructions
            i = 0
            while i < len(insts):
                inst = insts[i]
                si = inst.sync_info
                if si is not None and si.on_wait and len(si.on_wait) > max_waits:
                    waits = list(si.on_wait)
                    keep = waits[-max_waits:]
                    for k, w in enumerate(waits[:-max_waits]):
                        nop = mybir.InstNoOp(
                            name=f"{inst.name}_hoistwait_{k}",
                            sync_info=mybir.SyncInfo(on_wait=[w], on_update=[]),
                            bass_nofuse=True,
                            engine=inst.engine,
                        )
                        insts.insert(i, nop)
                        i += 1
                    si.on_wait = keep
                i += 1


def _build_program():
    """One SPMD program: per-core inputs
      tissue (64,256,256) bf16, cell (128,16,17,66) fp8e4 polyphase row
      quarter, w6t (36,128,64) fp8e4 x2048, bias (64,1) f32,
      roff (1,2) i32 = [row0, col0]
    output: out (64,256,256) bf16 = tissue with avg added in the ROI."""
    if "nc" in _CACHE:
        return _CACHE["nc"]
    bass, mybir, tile, _ = _get_modules()
    import bass_rust
    f32, bf16, i32 = mybir.dt.float32, mybir.dt.bfloat16, mybir.dt.int32
    fp8 = mybir.dt.float8e4

    nc = bass.Bass("TRN2", target_bir_lowering=False, debug=False,
                   num_devices=NCORES)
    tissue_d = nc.dram_tensor("tissue", (CH, H, W), bf16, kind="ExternalInput").ap()
    cell_d = nc.dram_tensor("cell", (C, PHASES, PRR, PRC), fp8,
                            kind="ExternalInput").ap()
    w6t_d = nc.dram_tensor("w6t", (36, C, CH), fp8, kind="ExternalInput").ap()
    bias_d = nc.dram_tensor("bias", (CH, 1), f32, kind="ExternalInput").ap()
    roff_d = nc.dram_tensor("roff", (1, 2), i32, kind="ExternalInput").ap()
    out_d = nc.dram_tensor("out", (CH, H, W), bf16, kind="ExternalOutput").ap()
    # avg exchange: own quarter out, gathered quarters in
    avgq_d = nc.dram_tensor("avgq", (CH, LBLK * ORB * ROI), bf16,
                            kind="Internal").ap()
    avgall_d = nc.dram_tensor("avgall", (4, CH, LBLK * ORB * ROI), bf16,
                              kind="Internal").ap()

    # taps grouped by polyphase plane so matmuls chase the plane DMAs
    tap_order = []       # (tap_idx, plane, row_shift, col_shift)
    for pp in range(4):
        for qq in range(4):
            for p in range(pp, 6, 4):
                for q in range(qq, 6, 4):
                    tap_order.append((p * 6 + q, pp * 4 + qq, p // 4, q // 4))
    assert len(tap_order) == 36

    with tile.TileContext(nc) as tc:
        with (
            tc.tile_pool(name="const", bufs=1) as constp,
            tc.tile_pool(name="cellp", bufs=1) as cellp,
            tc.tile_pool(name="roip", bufs=1) as roip,
            tc.tile_pool(name="psum", bufs=1, space="PSUM") as psump,
        ):
            # --- constants (scalar-engine DGE queue; tiny, first) ---
            roff_sb = constp.tile([1, 2], i32)
            nc.scalar.dma_start(roff_sb[:], roff_d[:])
            w_sb = constp.tile([C, 36 * CH], fp8)
            # w6t (36, C, CH) -> partition=input channel, free=(tap, out ch)
            nc.scalar.dma_start(w_sb[:], w6t_d.rearrange("t i o -> i t o"))
            bias_sb = constp.tile([CH, 1], f32)
            nc.scalar.dma_start(bias_sb[:], bias_d[:])

            # offsets are in-bounds by construction; the runtime assert's
            # ISA op miscompiles on this walrus build
            dyn_engines = (mybir.EngineType.SP, mybir.EngineType.Activation,
                           mybir.EngineType.Pool)
            r_v = nc.values_load(roff_sb[0:1, 0:1], engines=dyn_engines,
                                 min_val=0, max_val=H - ROI,
                                 skip_runtime_bounds_check=True)
            c_v = nc.values_load(roff_sb[0:1, 1:2], engines=dyn_engines,
                                 min_val=0, max_val=W - ROI,
                                 skip_runtime_bounds_check=True)

            # ROI source pixels: load early, overlaps with everything below
            roi_sb = roip.tile([CH, ROI * ROI], bf16)
            nc.gpsimd.dma_start(
                roi_sb[:], tissue_d[:, bass.ds(r_v, ROI), bass.ds(c_v, ROI)]
            )

            # --- polyphase cell plane quarters, resident in SBUF. One DMA
            # per plane: the SDMA round-robins across queue lanes, so small
            # per-plane units complete early and the matmuls chase them. ---
            cell_t = cellp.tile([C, PHASES * PRR * PRC], fp8)
            c4 = cell_t.rearrange("c (ph r w) -> c ph r w", r=PRR, w=PRC)
            for ph in range(PHASES):
                nc.sync.dma_start(c4[:, ph], cell_d[:, ph])

            # --- bulk copy tissue -> out: DRAM->DRAM, 128 KB contiguous
            # descriptors, no SBUF transit. Two channel-halves so the first
            # half of the dynamic ROI scatter overlaps the second half. ---
            t_flat = tissue_d.rearrange("c h w -> c (h w)")
            o_flat = out_d.rearrange("c h w -> c (h w)")
            CHH = CH // 2
            nc.sync.dma_start(o_flat[:CHH], t_flat[:CHH])
            nc.sync.dma_start(o_flat[CHH:], t_flat[CHH:])

            # roi_sb += bias (the gathered avg quarters carry no bias)
            nc.vector.tensor_scalar_add(roi_sb[:], roi_sb[:], bias_sb[:])

            # --- conv: 36 taps as 18 fp8 DoubleRow pairs over the 2 local
            # row blocks, accumulating in 2 PSUM banks. Each DoubleRow pass
            # contracts two taps at once via overlapping custom-stride APs.
            def _pair_ap(base, delta):
                ap = base.copy()
                dims = ap.ap.to_list()
                ap.ap = bass_rust.VecI64Pair(
                    [dims[0], [delta, 2]] + dims[1:])
                return ap

            pairs = []   # (t1, t2, ph1, pb1, qb1, cell_delta)
            for k in range(0, 36, 2):
                t1, ph1, pb1, qb1 = tap_order[k]
                t2, ph2, pb2, qb2 = tap_order[k + 1]
                dc = ((ph2 - ph1) * PRR * PRC + (pb2 - pb1) * PRC
                      + (qb2 - qb1))
                assert dc > 0
                pairs.append((t1, t2, ph1, pb1, qb1, dc))

            pss = [psump.tile([CH, ORB * ROI], f32, name=f"bank{b}")
                   for b in range(LBLK)]
            for i, (t1, t2, ph, pb, qb, dc) in enumerate(pairs):
                dw = (t2 - t1) * CH
                for b in range(LBLK):
                    nc.tensor.matmul(
                        pss[b][:],
                        _pair_ap(w_sb[:, t1 * CH:(t1 + 1) * CH], dw),
                        _pair_ap(
                            c4[:, ph, b * ORB + pb:b * ORB + pb + ORB,
                               qb:qb + ROI], dc),
                        start=(i == 0),
                        stop=(i == 17),
                        perf_mode=mybir.MatmulPerfMode.DoubleRow,
                    )

            # own avg quarter: descale fp8 weight prescale, to bf16
            avgq_sb = roip.tile([CH, LBLK * ORB * ROI], bf16, name="avgq")
            for b in range(LBLK):
                nc.vector.tensor_scalar_mul(
                    avgq_sb[:, b * ORB * ROI:(b + 1) * ORB * ROI],
                    pss[b][:], 1.0 / WS)
            # publish and exchange within the 4-core channel-half group
            nc.scalar.dma_start(avgq_d[:], avgq_sb[:])
            nc.gpsimd.collective_compute(
                "AllGather",
                mybir.AluOpType.bypass,
                replica_groups=[[0, 1, 2, 3], [4, 5, 6, 7]],
                ins=[avgq_d[:]],
                outs=[avgall_d[:]],
            )
            # gathered quarters, quarter-major == row-block-major: full avg
            avg_sb = roip.tile([CH, NBLK * ORB * ROI], bf16, name="avgfull")
            nc.scalar.dma_start(avg_sb.rearrange("c (q s) -> c q s", q=4),
                                avgall_d.rearrange("q c s -> c q s"))
            # roi += avg (single fused DVE pass)
            nc.vector.scalar_tensor_tensor(
                roi_sb[:], avg_sb[:], 1.0, roi_sb[:],
                mybir.AluOpType.mult, mybir.AluOpType.add,
            )

            # --- ROI scatter: overwrite after the bulk writes landed.
            # Each write engine stalls on the copy through a RAW hazard (a
            # 1-element read of the copied half); the scatter for channels
            # [0:32] starts as soon as copy half A lands, overlapping copy
            # half B; [32:64] follows half B. ---
            dummy = roip.tile([1, 8], bf16, name="wwsync")
            roi_dst = out_d[:, bass.ds(r_v, ROI), bass.ds(c_v, ROI)]
            splits = [(nc.sync, 0, 11, 32, 43, 0), (nc.scalar, 11, 21, 43, 53, 2),
                      (nc.gpsimd, 21, 32, 53, 64, 4)]
            for eng, a0, a1, b0, b1, i in splits:
                eng.dma_start(dummy[0:1, i:i + 1], o_flat[0:1, i:i + 1])
                eng.dma_start(roi_dst[a0:a1], roi_sb[a0:a1])
                eng.dma_start(dummy[0:1, i + 1:i + 2], o_flat[CHH:CHH + 1, i:i + 1])
                eng.dma_start(roi_dst[b0:b1], roi_sb[b0:b1])

    _split_multiwaits(nc, mybir)
    _CACHE["nc"] = nc
    return nc


def _prep_inputs(tissue_features, cell_features, loc, conv_w, conv_b):
    import ml_dtypes

    bf16 = ml_dtypes.bfloat16
    fp8 = ml_dtypes.float8_e4m3
    # fold AvgPool4x4 into the conv kernel: 6x6 taps
    w6 = np.zeros((C, C, 6, 6), np.float32)
    for dr in range(4):
        for dc in range(4):
            w6[:, :, dr:dr + 3, dc:dc + 3] += conv_w
    w6 *= WS / 16.0

    # polyphase split of the zero-padded cell map:
    # plane (pp,qq)[y,x] = padded[4y+pp, 4x+qq], padded = 1px zero border
    FPR = 65  # full polyphase plane rows
    padc = np.zeros((C, 4 * PRC, 4 * PRC), np.float32)
    padc[:, 1:1 + H, 1:1 + W] = cell_features[0]
    cell_poly = np.empty((C, PHASES, FPR, PRC), np.float32)
    for pp in range(4):
        for qq in range(4):
            cell_poly[:, pp * 4 + qq] = padc[:, pp:pp + 4 * FPR:4, qq::4]
    cell_poly = np.clip(cell_poly, -240, 240).astype(fp8)
    # per-sample row quarter (+1 halo row), aligned so local rows 0..16
    # cover pooled rows 16j..16j+15
    cell_q = [np.ascontiguousarray(cell_poly[:, :, 16 * j:16 * j + PRR])
              for j in range(4)]

    w6t = {}
    bias = {}
    for h in range(2):
        sl = slice(CH * h, CH * (h + 1))
        # (CH, C, 6, 6) -> (tap, in ch, out ch)
        w6t[h] = np.ascontiguousarray(
            np.clip(w6[sl].transpose(2, 3, 1, 0).reshape(36, C, CH), -240, 240)
        ).astype(fp8)
        bias[h] = np.ascontiguousarray(conv_b[sl].astype(np.float32)).reshape(CH, 1)

    r0 = loc[:, 1].astype(np.int64) * W // 1024 - L   # H-dim start (from loc x)
    c0 = loc[:, 0].astype(np.int64) * W // 1024 - L   # W-dim start (from loc y)

    tissue_bf = tissue_features.astype(bf16)
    in_maps = []
    for c in range(NCORES):
        j, h = c % B, c // B
        in_maps.append({
            "tissue": tissue_bf[j, CH * h:CH * (h + 1)],
            "cell": cell_q[j],
            "w6t": w6t[h],
            "bias": bias[h],
            "roff": np.array([[r0[j], c0[j]]], np.int32),
        })
    return in_maps


def run_device(tissue_features, cell_features, loc, conv_w, conv_b, **spmd_kwargs):
    """Build+run the SPMD kernel; returns (final (4,128,256,256), raw results)."""
    *_, run_bass_kernel_spmd = _get_modules()
    nc = _build_program()
    in_maps = _prep_inputs(tissue_features, cell_features, loc, conv_w, conv_b)
    res = run_bass_kernel_spmd(nc, in_maps, list(range(NCORES)), **spmd_kwargs)
    final = np.empty((B, C, H, W), np.float32)
    for c in range(NCORES):
        j, h = c % B, c // B
        final[j, CH * h:CH * (h + 1)] = res.results[c]["out"].astype(np.float32)
    return final, res


def kernel(tissue_features, cell_features, loc, conv_w, conv_b):
    final, _ = run_device(tissue_features, cell_features, loc, conv_w, conv_b)
    # reference stacks B copies of the fully-mutated tissue
    return np.broadcast_to(final[None], (B, B, C, H, W))


# revision 18
# speedup vs baseline: 1.3718x; 1.3718x over previous
"""Trainium2 Bass kernel for nn_Cell2Tissue (scatter_memory).

Reference computation:
  avg = AvgPool4x4(Conv3x3_SAME(cell) + bias)          # (128, 64, 64)
  for each tissue sample j: ROI_j += avg               # 64x64 ROI from loc
  output = stack of B copies of the mutated tissue     # (4, 4, 128, 256, 256)

Sharding over 8 cores: core c = (sample j = c % 4, channel half h = c // 4).

v2 strategy (memory-bound; tolerance gate is rel_err < 2e-2):
- tissue/out streamed as bf16 (host converts; ~0.23% rel err) -> bulk
  copy bytes halve to 16.8 MB/core.
- cell polyphase planes and folded 6x6 weights in fp8e4 (weights
  pre-scaled x2048, descaled in the DVE add) -> 8.8 MB/core replicated.
  Simulated end-to-end rel err 2.3e-3.
- bulk copy is a single DRAM->DRAM DMA (64 descriptors x 128 KB
  contiguous) instead of 32 strided through-SBUF tile DMAs.
- static-ring order: constants, cell planes, bulk copy - so matmuls
  chase the plane loads and finish under the copy.
- dynamic ROI read early on gpsimd; ROI write after an all-engine
  barrier, split across the 3 dynamic-capable queues (sync/scalar/
  gpsimd).
"""

import os
import numpy as np

B, C, H, W = 4, 128, 256, 256
CH = C // 2          # channels per core (half)
L = 32               # half ROI width
ROI = 2 * L          # 64
NCORES = 8
PRR = 65             # polyphase plane rows (max y+pb = 64)
PRC = 66             # polyphase plane cols
PHASES = 16
WS = 2048.0          # fp8 weight pre-scale (power of 2; descaled on DVE)

_CACHE = {}


def _get_modules():
    if "mods" in _CACHE:
        return _CACHE["mods"]
    # bass2jax executes via the jax 'axon'/'neuron' platform; a cpu-pinned
    # JAX_PLATFORMS would hide the devices.
    if os.environ.get("JAX_PLATFORMS") in ("cpu",):
        del os.environ["JAX_PLATFORMS"]
    import concourse.bass as bass
    import concourse.mybir as mybir
    import concourse.tile as tile
    from concourse.bass_utils import run_bass_kernel_spmd

    _CACHE["mods"] = (bass, mybir, tile, run_bass_kernel_spmd)
    return _CACHE["mods"]


def _split_multiwaits(nc, mybir, max_waits=1):
    """The walrus build here rejects >1 sem-wait on some instructions (the
    Tile tail InstDrain). Hoist extra waits onto single-wait nops placed
    immediately before, on the same engine (same-engine program order
    preserves semantics)."""
    for fn in nc.m.functions:
        for bb in fn.blocks:
            insts = bb.instructions
            i = 0
            while i < len(insts):
                inst = insts[i]
                si = inst.sync_info
                if si is not None and si.on_wait and len(si.on_wait) > max_waits:
                    waits = list(si.on_wait)
                    keep = waits[-max_waits:]
                    for k, w in enumerate(waits[:-max_waits]):
                        nop = mybir.InstNoOp(
                            name=f"{inst.name}_hoistwait_{k}",
                            sync_info=mybir.SyncInfo(on_wait=[w], on_update=[]),
                            bass_nofuse=True,
                            engine=inst.engine,
                        )
                        insts.insert(i, nop)
                        i += 1
                    si.on_wait = keep
                i += 1


def _build_program():
    """One SPMD program: per-core inputs
      tissue (64,256,256) bf16, cell (128,16,65,66) fp8e4 polyphase
      (replicated), w6t (36,128,64) fp8e4 x2048, bias (64,1) f32,
      roff (1,2) i32 = [row0, col0]
    output: out (64,256,256) bf16 = tissue with avg added in the ROI."""
    if "nc" in _CACHE:
        return _CACHE["nc"]
    bass, mybir, tile, _ = _get_modules()
    f32, bf16, i32 = mybir.dt.float32, mybir.dt.bfloat16, mybir.dt.int32
    fp8 = mybir.dt.float8e4

    nc = bass.Bass("TRN2", target_bir_lowering=False, debug=False,
                   num_devices=NCORES)
    tissue_d = nc.dram_tensor("tissue", (CH, H, W), bf16, kind="ExternalInput").ap()
    cell_d = nc.dram_tensor("cell", (C, PHASES, PRR, PRC), fp8,
                            kind="ExternalInput").ap()
    w6t_d = nc.dram_tensor("w6t", (36, C, CH), fp8, kind="ExternalInput").ap()
    bias_d = nc.dram_tensor("bias", (CH, 1), f32, kind="ExternalInput").ap()
    roff_d = nc.dram_tensor("roff", (1, 2), i32, kind="ExternalInput").ap()
    out_d = nc.dram_tensor("out", (CH, H, W), bf16, kind="ExternalOutput").ap()

    NBLK = 8             # conv row blocks: 8 output rows, one PSUM bank each
    ORB = ROI // NBLK    # 8 output rows per block

    # taps grouped by polyphase plane so matmuls chase the plane DMAs
    tap_order = []       # (tap_idx, plane, row_shift, col_shift)
    for pp in range(4):
        for qq in range(4):
            for p in range(pp, 6, 4):
                for q in range(qq, 6, 4):
                    tap_order.append((p * 6 + q, pp * 4 + qq, p // 4, q // 4))
    assert len(tap_order) == 36

    with tile.TileContext(nc) as tc:
        with (
            tc.tile_pool(name="const", bufs=1) as constp,
            tc.tile_pool(name="cellp", bufs=1) as cellp,
            tc.tile_pool(name="roip", bufs=1) as roip,
            tc.tile_pool(name="psum", bufs=1, space="PSUM") as psump,
        ):
            # --- constants (scalar-engine DGE queue; tiny, first) ---
            roff_sb = constp.tile([1, 2], i32)
            nc.scalar.dma_start(roff_sb[:], roff_d[:])
            w_sb = constp.tile([C, 36 * CH], fp8)
            # w6t (36, C, CH) -> partition=input channel, free=(tap, out ch)
            nc.scalar.dma_start(w_sb[:], w6t_d.rearrange("t i o -> i t o"))
            bias_sb = constp.tile([CH, 1], f32)
            nc.scalar.dma_start(bias_sb[:], bias_d[:])

            # offsets are in-bounds by construction; the runtime assert's
            # ISA op miscompiles on this walrus build
            dyn_engines = (mybir.EngineType.SP, mybir.EngineType.Activation,
                           mybir.EngineType.Pool)
            r_v = nc.values_load(roff_sb[0:1, 0:1], engines=dyn_engines,
                                 min_val=0, max_val=H - ROI,
                                 skip_runtime_bounds_check=True)
            c_v = nc.values_load(roff_sb[0:1, 1:2], engines=dyn_engines,
                                 min_val=0, max_val=W - ROI,
                                 skip_runtime_bounds_check=True)

            # ROI source pixels: load early, overlaps with everything below
            roi_sb = roip.tile([CH, ROI * ROI], bf16)
            nc.gpsimd.dma_start(
                roi_sb[:], tissue_d[:, bass.ds(r_v, ROI), bass.ds(c_v, ROI)]
            )

            # --- polyphase cell planes, resident in SBUF. One DMA per
            # plane: the SDMA round-robins across queue lanes, so small
            # per-plane units complete early and the matmuls chase them
            # (grouped loads all complete late and stall the PE). ---
            cell_t = cellp.tile([C, PHASES * PRR * PRC], fp8)
            c4 = cell_t.rearrange("c (ph r w) -> c ph r w", r=PRR, w=PRC)
            for ph in range(PHASES):
                nc.sync.dma_start(c4[:, ph], cell_d[:, ph])

            # --- bulk copy tissue -> out: DRAM->DRAM, 128 KB contiguous
            # descriptors, no SBUF transit. Four 16-channel chunks, each
            # gating one phase of the dynamic ROI scatter so the scatter
            # (descriptor-rate-bound) hides under the copy. Chunks 0-2 ride
            # the scalar HWDGE ring (free early), chunk 3 follows the
            # planes on the sync ring. ---
            t_flat = tissue_d.rearrange("c h w -> c (h w)")
            o_flat = out_d.rearrange("c h w -> c (h w)")
            CQ = CH // 4
            for k, eng in enumerate((nc.scalar, nc.scalar, nc.scalar, nc.sync)):
                eng.dma_start(o_flat[k * CQ:(k + 1) * CQ],
                              t_flat[k * CQ:(k + 1) * CQ])

            # roi_sb += bias (frees the fused op below for the fp8 descale)
            nc.vector.tensor_scalar_add(roi_sb[:], roi_sb[:], bias_sb[:])

            # --- conv: 36 taps as 18 fp8 DoubleRow pairs x 8 blocks,
            # accumulating in 8 PSUM banks. Each DoubleRow pass contracts
            # two taps at once (2x PE throughput); the pair dim is an
            # overlapping custom-stride AP over the resident planes. ---
            import bass_rust

            def _pair_ap(base, delta):
                ap = base.copy()
                dims = ap.ap.to_list()
                ap.ap = bass_rust.VecI64Pair(
                    [dims[0], [delta, 2]] + dims[1:])
                return ap

            pairs = []   # (t1, t2, ph1, pb1, qb1, cell_delta)
            for k in range(0, 36, 2):
                t1, ph1, pb1, qb1 = tap_order[k]
                t2, ph2, pb2, qb2 = tap_order[k + 1]
                dc = ((ph2 - ph1) * PRR * PRC + (pb2 - pb1) * PRC
                      + (qb2 - qb1))
                assert dc > 0
                pairs.append((t1, t2, ph1, pb1, qb1, dc))

            pss = [psump.tile([CH, ORB * ROI], f32, name=f"bank{b}")
                   for b in range(NBLK)]
            for i, (t1, t2, ph, pb, qb, dc) in enumerate(pairs):
                dw = (t2 - t1) * CH
                for b in range(NBLK):
                    nc.tensor.matmul(
                        pss[b][:],
                        _pair_ap(w_sb[:, t1 * CH:(t1 + 1) * CH], dw),
                        _pair_ap(
                            c4[:, ph, b * ORB + pb:b * ORB + pb + ORB,
                               qb:qb + ROI], dc),
                        start=(i == 0),
                        stop=(i == 17),
                        perf_mode=mybir.MatmulPerfMode.DoubleRow,
                    )
            # roi strip b += psum[b]/WS  (fused on DVE)
            for b in range(NBLK):
                strip = slice(b * ORB * ROI, (b + 1) * ORB * ROI)
                nc.vector.scalar_tensor_tensor(
                    roi_sb[:, strip], pss[b][:], 1.0 / WS, roi_sb[:, strip],
                    mybir.AluOpType.mult, mybir.AluOpType.add,
                )

            # --- ROI scatter: overwrite after the bulk writes landed.
            # Instead of a full-engine barrier (expensive staggered sem
            # chains), each write engine stalls on copy chunk k through a
            # RAW hazard (a 1-element read of the chunk), then scatters
            # that chunk's channels. Phases k=0..2 overlap later copy
            # chunks; only phase 3's writes trail the copy. ---
            dummy = roip.tile([1, 16], bf16, name="wwsync")
            roi_dst = out_d[:, bass.ds(r_v, ROI), bass.ds(c_v, ROI)]
            engs = (nc.sync, nc.scalar, nc.gpsimd)
            for k in range(4):
                base = k * CQ
                cuts = (base, base + 6, base + 11, base + CQ)
                for e, eng in enumerate(engs):
                    eng.dma_start(dummy[0:1, 3 * k + e:3 * k + e + 1],
                                  o_flat[base:base + 1, e:e + 1])
                    eng.dma_start(roi_dst[cuts[e]:cuts[e + 1]],
                                  roi_sb[cuts[e]:cuts[e + 1]])

    _split_multiwaits(nc, mybir)
    _CACHE["nc"] = nc
    return nc


def _prep_inputs(tissue_features, cell_features, loc, conv_w, conv_b):
    import ml_dtypes

    bf16 = ml_dtypes.bfloat16
    fp8 = ml_dtypes.float8_e4m3
    # fold AvgPool4x4 into the conv kernel: 6x6 taps
    w6 = np.zeros((C, C, 6, 6), np.float32)
    for dr in range(4):
        for dc in range(4):
            w6[:, :, dr:dr + 3, dc:dc + 3] += conv_w
    w6 *= WS / 16.0

    # polyphase split of the zero-padded cell map:
    # plane (pp,qq)[y,x] = padded[4y+pp, 4x+qq], padded = 1px zero border
    padc = np.zeros((C, 4 * PRC, 4 * PRC), np.float32)
    padc[:, 1:1 + H, 1:1 + W] = cell_features[0]
    cell_poly = np.empty((C, PHASES, PRR, PRC), np.float32)
    for pp in range(4):
        for qq in range(4):
            cell_poly[:, pp * 4 + qq] = padc[:, pp:pp + 4 * PRR:4, qq::4]
    cell_poly = np.ascontiguousarray(np.clip(cell_poly, -240, 240)).astype(fp8)

    w6t = {}
    bias = {}
    for h in range(2):
        sl = slice(CH * h, CH * (h + 1))
        # (CH, C, 6, 6) -> (tap, in ch, out ch)
        w6t[h] = np.ascontiguousarray(
            np.clip(w6[sl].transpose(2, 3, 1, 0).reshape(36, C, CH), -240, 240)
        ).astype(fp8)
        bias[h] = np.ascontiguousarray(conv_b[sl].astype(np.float32)).reshape(CH, 1)

    r0 = loc[:, 1].astype(np.int64) * W // 1024 - L   # H-dim start (from loc x)
    c0 = loc[:, 0].astype(np.int64) * W // 1024 - L   # W-dim start (from loc y)

    tissue_bf = tissue_features.astype(bf16)
    in_maps = []
    for c in range(NCORES):
        j, h = c % B, c // B
        in_maps.append({
            "tissue": tissue_bf[j, CH * h:CH * (h + 1)],
            "cell": cell_poly,
            "w6t": w6t[h],
            "bias": bias[h],
            "roff": np.array([[r0[j], c0[j]]], np.int32),
        })
    return in_maps


def run_device(tissue_features, cell_features, loc, conv_w, conv_b, **spmd_kwargs):
    """Build+run the SPMD kernel; returns (final (4,128,256,256), raw results)."""
    *_, run_bass_kernel_spmd = _get_modules()
    nc = _build_program()
    in_maps = _prep_inputs(tissue_features, cell_features, loc, conv_w, conv_b)
    res = run_bass_kernel_spmd(nc, in_maps, list(range(NCORES)), **spmd_kwargs)
    final = np.empty((B, C, H, W), np.float32)
    for c in range(NCORES):
        j, h = c % B, c // B
        final[j, CH * h:CH * (h + 1)] = res.results[c]["out"].astype(np.float32)
    return final, res


def kernel(tissue_features, cell_features, loc, conv_w, conv_b):
    final, _ = run_device(tissue_features, cell_features, loc, conv_w, conv_b)
    # reference stacks B copies of the fully-mutated tissue
    return np.broadcast_to(final[None], (B, B, C, H, W))


# revision 28
# speedup vs baseline: 1.4315x; 1.0435x over previous
"""Trainium2 Bass kernel for nn_Cell2Tissue (scatter_memory).

Reference computation:
  avg = AvgPool4x4(Conv3x3_SAME(cell) + bias)          # (128, 64, 64)
  for each tissue sample j: ROI_j += avg               # 64x64 ROI from loc
  output = stack of B copies of the mutated tissue     # (4, 4, 128, 256, 256)

Sharding over 8 cores: core c = (sample j = c % 4, channel half h = c // 4).

v2 strategy (memory-bound; tolerance gate is rel_err < 2e-2):
- tissue/out streamed as bf16 (host converts; ~0.23% rel err) -> bulk
  copy bytes halve to 16.8 MB/core.
- cell polyphase planes and folded 6x6 weights in fp8e4 (weights
  pre-scaled x2048, descaled in the DVE add) -> 8.8 MB/core replicated.
  Simulated end-to-end rel err 2.3e-3.
- bulk copy is a single DRAM->DRAM DMA (64 descriptors x 128 KB
  contiguous) instead of 32 strided through-SBUF tile DMAs.
- static-ring order: constants, cell planes, bulk copy - so matmuls
  chase the plane loads and finish under the copy.
- dynamic ROI read early on gpsimd; ROI write after an all-engine
  barrier, split across the 3 dynamic-capable queues (sync/scalar/
  gpsimd).
"""

import os
import numpy as np

B, C, H, W = 4, 128, 256, 256
CH = C // 2          # channels per core (half)
L = 32               # half ROI width
ROI = 2 * L          # 64
NCORES = 8
PRR = 65             # polyphase plane rows (max y+pb = 64)
PRC = 66             # polyphase plane cols
PHASES = 16
WS = 2048.0          # fp8 weight pre-scale (power of 2; descaled on DVE)

_CACHE = {}


def _get_modules():
    if "mods" in _CACHE:
        return _CACHE["mods"]
    # bass2jax executes via the jax 'axon'/'neuron' platform; a cpu-pinned
    # JAX_PLATFORMS would hide the devices.
    if os.environ.get("JAX_PLATFORMS") in ("cpu",):
        del os.environ["JAX_PLATFORMS"]
    import concourse.bass as bass
    import concourse.mybir as mybir
    import concourse.tile as tile
    from concourse.bass_utils import run_bass_kernel_spmd

    _CACHE["mods"] = (bass, mybir, tile, run_bass_kernel_spmd)
    return _CACHE["mods"]


def _split_multiwaits(nc, mybir, max_waits=1):
    """The walrus build here rejects >1 sem-wait on some instructions (the
    Tile tail InstDrain). Hoist extra waits onto single-wait nops placed
    immediately before, on the same engine (same-engine program order
    preserves semantics)."""
    for fn in nc.m.functions:
        for bb in fn.blocks:
            insts = bb.instructions
            i = 0
            while i < len(insts):
                inst = insts[i]
                si = inst.sync_info
                if si is not None and si.on_wait and len(si.on_wait) > max_waits:
                    waits = list(si.on_wait)
                    keep = waits[-max_waits:]
                    for k, w in enumerate(waits[:-max_waits]):
                        nop = mybir.InstNoOp(
                            name=f"{inst.name}_hoistwait_{k}",
                            sync_info=mybir.SyncInfo(on_wait=[w], on_update=[]),
                            bass_nofuse=True,
                            engine=inst.engine,
                        )
                        insts.insert(i, nop)
                        i += 1
                    si.on_wait = keep
                i += 1


def _build_program():
    """One SPMD program: per-core inputs
      tissue (64,256,256) bf16, cell (128,16,65,66) fp8e4 polyphase
      (replicated), w6t (36,128,64) fp8e4 x2048, bias (64,1) f32,
      roff (1,2) i32 = [row0, col0]
    output: out (64,256,256) bf16 = tissue with avg added in the ROI."""
    if "nc" in _CACHE:
        return _CACHE["nc"]
    bass, mybir, tile, _ = _get_modules()
    f32, bf16, i32 = mybir.dt.float32, mybir.dt.bfloat16, mybir.dt.int32
    fp8 = mybir.dt.float8e4

    nc = bass.Bass("TRN2", target_bir_lowering=False, debug=False,
                   num_devices=NCORES)
    tissue_d = nc.dram_tensor("tissue", (CH, H, W), bf16, kind="ExternalInput").ap()
    cell_d = nc.dram_tensor("cell", (C, PHASES, PRR, PRC), fp8,
                            kind="ExternalInput").ap()
    w6t_d = nc.dram_tensor("w6t", (36, C, CH), fp8, kind="ExternalInput").ap()
    bias_d = nc.dram_tensor("bias", (CH, 1), f32, kind="ExternalInput").ap()
    roff_d = nc.dram_tensor("roff", (1, 2), i32, kind="ExternalInput").ap()
    out_d = nc.dram_tensor("out", (CH, H, W), bf16, kind="ExternalOutput").ap()

    NBLK = 8             # conv row blocks: 8 output rows, one PSUM bank each
    ORB = ROI // NBLK    # 8 output rows per block

    # taps grouped by polyphase plane so matmuls chase the plane DMAs
    tap_order = []       # (tap_idx, plane, row_shift, col_shift)
    for pp in range(4):
        for qq in range(4):
            for p in range(pp, 6, 4):
                for q in range(qq, 6, 4):
                    tap_order.append((p * 6 + q, pp * 4 + qq, p // 4, q // 4))
    assert len(tap_order) == 36

    with tile.TileContext(nc) as tc:
        with (
            tc.tile_pool(name="const", bufs=1) as constp,
            tc.tile_pool(name="cellp", bufs=1) as cellp,
            tc.tile_pool(name="roip", bufs=1) as roip,
            tc.tile_pool(name="psum", bufs=1, space="PSUM") as psump,
        ):
            # --- constants (scalar-engine DGE queue; tiny, first) ---
            roff_sb = constp.tile([1, 2], i32)
            nc.scalar.dma_start(roff_sb[:], roff_d[:])
            w_sb = constp.tile([C, 36 * CH], fp8)
            # w6t (36, C, CH) -> partition=input channel, free=(tap, out ch)
            nc.scalar.dma_start(w_sb[:], w6t_d.rearrange("t i o -> i t o"))
            bias_sb = constp.tile([CH, 1], f32)
            nc.scalar.dma_start(bias_sb[:], bias_d[:])

            # offsets are in-bounds by construction; the runtime assert's
            # ISA op miscompiles on this walrus build
            dyn_engines = (mybir.EngineType.SP, mybir.EngineType.Activation,
                           mybir.EngineType.Pool)
            r_v = nc.values_load(roff_sb[0:1, 0:1], engines=dyn_engines,
                                 min_val=0, max_val=H - ROI,
                                 skip_runtime_bounds_check=True)
            c_v = nc.values_load(roff_sb[0:1, 1:2], engines=dyn_engines,
                                 min_val=0, max_val=W - ROI,
                                 skip_runtime_bounds_check=True)

            # ROI source pixels: load early, overlaps with everything below
            roi_sb = roip.tile([CH, ROI * ROI], bf16)
            nc.gpsimd.dma_start(
                roi_sb[:], tissue_d[:, bass.ds(r_v, ROI), bass.ds(c_v, ROI)]
            )

            # --- polyphase cell planes, resident in SBUF. One DMA per
            # plane: the SDMA round-robins across queue lanes, so small
            # per-plane units complete early and the matmuls chase them
            # (grouped loads all complete late and stall the PE). ---
            cell_t = cellp.tile([C, PHASES * PRR * PRC], fp8)
            c4 = cell_t.rearrange("c (ph r w) -> c ph r w", r=PRR, w=PRC)
            for ph in range(PHASES):
                nc.sync.dma_start(c4[:, ph], cell_d[:, ph])

            # --- bulk copy tissue -> out: DRAM->DRAM, 128 KB contiguous
            # descriptors, no SBUF transit. Half A follows the planes on
            # the sync ring; half B rides the scalar ring, gated behind the
            # last plane (scalar stalls on a register read of it) so the
            # two halves drain concurrently WITHOUT starving the planes
            # that the matmuls chase. ---
            t_flat = tissue_d.rearrange("c h w -> c (h w)")
            o_flat = out_d.rearrange("c h w -> c (h w)")
            CHH = CH // 2
            nc.sync.dma_start(o_flat[:CHH], t_flat[:CHH])
            nc.values_load(cell_t[0:1, (PHASES - 1) * PRR * PRC + 2:
                                  (PHASES - 1) * PRR * PRC + 6].bitcast(i32),
                           engines=(mybir.EngineType.Activation,),
                           skip_runtime_bounds_check=True)
            nc.scalar.dma_start(o_flat[CHH:], t_flat[CHH:])

            # roi_sb += bias (frees the fused op below for the fp8 descale)
            nc.vector.tensor_scalar_add(roi_sb[:], roi_sb[:], bias_sb[:])

            # --- conv: 36 taps as 18 fp8 DoubleRow pairs x 8 blocks,
            # accumulating in 8 PSUM banks. Each DoubleRow pass contracts
            # two taps at once (2x PE throughput); the pair dim is an
            # overlapping custom-stride AP over the resident planes. ---
            import bass_rust

            def _pair_ap(base, delta):
                ap = base.copy()
                dims = ap.ap.to_list()
                ap.ap = bass_rust.VecI64Pair(
                    [dims[0], [delta, 2]] + dims[1:])
                return ap

            pairs = []   # (t1, t2, ph1, pb1, qb1, cell_delta)
            for k in range(0, 36, 2):
                t1, ph1, pb1, qb1 = tap_order[k]
                t2, ph2, pb2, qb2 = tap_order[k + 1]
                dc = ((ph2 - ph1) * PRR * PRC + (pb2 - pb1) * PRC
                      + (qb2 - qb1))
                assert dc > 0
                pairs.append((t1, t2, ph1, pb1, qb1, dc))

            # matmul N is capped at one PSUM bank (512 f32), so 8 per-bank
            # matmuls per pair; walrus ldw-opt (enabled below) dedupes the
            # 8 identical LDWEIGHTS per pair
            pss = psump.tile([CH, NBLK * ORB * ROI], f32, name="acc")
            for i, (t1, t2, ph, pb, qb, dc) in enumerate(pairs):
                dw = (t2 - t1) * CH
                for b in range(NBLK):
                    nc.tensor.matmul(
                        pss[:, b * ORB * ROI:(b + 1) * ORB * ROI],
                        _pair_ap(w_sb[:, t1 * CH:(t1 + 1) * CH], dw),
                        _pair_ap(c4[:, ph, b * ORB + pb:b * ORB + pb + ORB,
                                    qb:qb + ROI], dc),
                        start=(i == 0),
                        stop=(i == 17),
                        perf_mode=mybir.MatmulPerfMode.DoubleRow,
                    )
            # roi strip b += psum strip b / WS  (fused on DVE; gpsimd
            # cannot access PSUM)
            for b in range(NBLK):
                strip = slice(b * ORB * ROI, (b + 1) * ORB * ROI)
                nc.vector.scalar_tensor_tensor(
                    roi_sb[:, strip], pss[:, strip], 1.0 / WS,
                    roi_sb[:, strip],
                    mybir.AluOpType.mult, mybir.AluOpType.add,
                )

            # --- ROI scatter: overwrite after the bulk writes landed.
            # Instead of a full-engine barrier (expensive staggered sem
            # chains), each write engine stalls on the copy through a RAW
            # hazard: a 1-element read of the copied half. The scatter for
            # channels [0:32] starts as soon as copy half A lands,
            # overlapping copy half B; [32:64] follows half B. ---
            dummy = roip.tile([1, 8], bf16, name="wwsync")
            roi_dst = out_d[:, bass.ds(r_v, ROI), bass.ds(c_v, ROI)]
            splits = [(nc.sync, 0, 11, 32, 43, 0), (nc.scalar, 11, 21, 43, 53, 2),
                      (nc.gpsimd, 21, 32, 53, 64, 4)]
            for eng, a0, a1, b0, b1, i in splits:
                eng.dma_start(dummy[0:1, i:i + 1], o_flat[0:1, i:i + 1])
                eng.dma_start(roi_dst[a0:a1], roi_sb[a0:a1])
                eng.dma_start(dummy[0:1, i + 1:i + 2], o_flat[CHH:CHH + 1, i:i + 1])
                eng.dma_start(roi_dst[b0:b1], roi_sb[b0:b1])

    _split_multiwaits(nc, mybir)
    _CACHE["nc"] = nc
    return nc


def _prep_inputs(tissue_features, cell_features, loc, conv_w, conv_b):
    import ml_dtypes

    bf16 = ml_dtypes.bfloat16
    fp8 = ml_dtypes.float8_e4m3
    # fold AvgPool4x4 into the conv kernel: 6x6 taps
    w6 = np.zeros((C, C, 6, 6), np.float32)
    for dr in range(4):
        for dc in range(4):
            w6[:, :, dr:dr + 3, dc:dc + 3] += conv_w
    w6 *= WS / 16.0

    # polyphase split of the zero-padded cell map:
    # plane (pp,qq)[y,x] = padded[4y+pp, 4x+qq], padded = 1px zero border
    padc = np.zeros((C, 4 * PRC, 4 * PRC), np.float32)
    padc[:, 1:1 + H, 1:1 + W] = cell_features[0]
    cell_poly = np.empty((C, PHASES, PRR, PRC), np.float32)
    for pp in range(4):
        for qq in range(4):
            cell_poly[:, pp * 4 + qq] = padc[:, pp:pp + 4 * PRR:4, qq::4]
    cell_poly = np.ascontiguousarray(np.clip(cell_poly, -240, 240)).astype(fp8)

    w6t = {}
    bias = {}
    for h in range(2):
        sl = slice(CH * h, CH * (h + 1))
        # (CH, C, 6, 6) -> (tap, in ch, out ch)
        w6t[h] = np.ascontiguousarray(
            np.clip(w6[sl].transpose(2, 3, 1, 0).reshape(36, C, CH), -240, 240)
        ).astype(fp8)
        bias[h] = np.ascontiguousarray(conv_b[sl].astype(np.float32)).reshape(CH, 1)

    r0 = loc[:, 1].astype(np.int64) * W // 1024 - L   # H-dim start (from loc x)
    c0 = loc[:, 0].astype(np.int64) * W // 1024 - L   # W-dim start (from loc y)

    tissue_bf = tissue_features.astype(bf16)
    in_maps = []
    for c in range(NCORES):
        j, h = c % B, c // B
        in_maps.append({
            "tissue": tissue_bf[j, CH * h:CH * (h + 1)],
            "cell": cell_poly,
            "w6t": w6t[h],
            "bias": bias[h],
            "roff": np.array([[r0[j], c0[j]]], np.int32),
        })
    return in_maps


def run_device(tissue_features, cell_features, loc, conv_w, conv_b, **spmd_kwargs):
    """Build+run the SPMD kernel; returns (final (4,128,256,256), raw results)."""
    *_, run_bass_kernel_spmd = _get_modules()
    nc = _build_program()
    in_maps = _prep_inputs(tissue_features, cell_features, loc, conv_w, conv_b)
    res = run_bass_kernel_spmd(nc, in_maps, list(range(NCORES)), **spmd_kwargs)
    final = np.empty((B, C, H, W), np.float32)
    for c in range(NCORES):
        j, h = c % B, c // B
        final[j, CH * h:CH * (h + 1)] = res.results[c]["out"].astype(np.float32)
    return final, res


def kernel(tissue_features, cell_features, loc, conv_w, conv_b):
    final, _ = run_device(tissue_features, cell_features, loc, conv_w, conv_b)
    # reference stacks B copies of the fully-mutated tissue
    return np.broadcast_to(final[None], (B, B, C, H, W))


# revision 29
# speedup vs baseline: 1.5706x; 1.0972x over previous
"""Trainium2 Bass kernel for nn_Cell2Tissue (scatter_memory).

Reference computation:
  avg = AvgPool4x4(Conv3x3_SAME(cell) + bias)          # (128, 64, 64)
  for each tissue sample j: ROI_j += avg               # 64x64 ROI from loc
  output = stack of B copies of the mutated tissue     # (4, 4, 128, 256, 256)

Sharding over 8 cores: core c = (sample j = c % 4, channel half h = c // 4).

v2 strategy (memory-bound; tolerance gate is rel_err < 2e-2):
- tissue/out streamed as bf16 (host converts; ~0.23% rel err) -> bulk
  copy bytes halve to 16.8 MB/core.
- cell polyphase planes and folded 6x6 weights in fp8e4 (weights
  pre-scaled x2048, descaled in the DVE add) -> 8.8 MB/core replicated.
  Simulated end-to-end rel err 2.3e-3.
- bulk copy is a single DRAM->DRAM DMA (64 descriptors x 128 KB
  contiguous) instead of 32 strided through-SBUF tile DMAs.
- static-ring order: constants, cell planes, bulk copy - so matmuls
  chase the plane loads and finish under the copy.
- dynamic ROI read early on gpsimd; ROI write after an all-engine
  barrier, split across the 3 dynamic-capable queues (sync/scalar/
  gpsimd).
"""

import os
import numpy as np

B, C, H, W = 4, 128, 256, 256
CH = C // 2          # channels per core (half)
L = 32               # half ROI width
ROI = 2 * L          # 64
NCORES = 8
PRR = 65             # polyphase plane rows (max y+pb = 64)
PRC = 66             # polyphase plane cols
PHASES = 16
WS = 2048.0          # fp8 weight pre-scale (power of 2; descaled on DVE)

_CACHE = {}


def _get_modules():
    if "mods" in _CACHE:
        return _CACHE["mods"]
    # bass2jax executes via the jax 'axon'/'neuron' platform; a cpu-pinned
    # JAX_PLATFORMS would hide the devices.
    if os.environ.get("JAX_PLATFORMS") in ("cpu",):
        del os.environ["JAX_PLATFORMS"]
    import concourse.bass as bass
    import concourse.mybir as mybir
    import concourse.tile as tile
    from concourse.bass_utils import run_bass_kernel_spmd

    _CACHE["mods"] = (bass, mybir, tile, run_bass_kernel_spmd)
    return _CACHE["mods"]


def _split_multiwaits(nc, mybir, max_waits=1):
    """The walrus build here rejects >1 sem-wait on some instructions (the
    Tile tail InstDrain). Hoist extra waits onto single-wait nops placed
    immediately before, on the same engine (same-engine program order
    preserves semantics)."""
    for fn in nc.m.functions:
        for bb in fn.blocks:
            insts = bb.instructions
            i = 0
            while i < len(insts):
                inst = insts[i]
                si = inst.sync_info
                if si is not None and si.on_wait and len(si.on_wait) > max_waits:
                    waits = list(si.on_wait)
                    keep = waits[-max_waits:]
                    for k, w in enumerate(waits[:-max_waits]):
                        nop = mybir.InstNoOp(
                            name=f"{inst.name}_hoistwait_{k}",
                            sync_info=mybir.SyncInfo(on_wait=[w], on_update=[]),
                            bass_nofuse=True,
                            engine=inst.engine,
                        )
                        insts.insert(i, nop)
                        i += 1
                    si.on_wait = keep
                i += 1


def _build_program():
    """One SPMD program: per-core inputs
      tissue (64,256,256) bf16, cell (128,16,65,66) fp8e4 polyphase
      (replicated), w6t (36,128,64) fp8e4 x2048, bias (64,1) f32,
      roff (1,2) i32 = [row0, col0]
    output: out (64,256,256) bf16 = tissue with avg added in the ROI."""
    if "nc" in _CACHE:
        return _CACHE["nc"]
    bass, mybir, tile, _ = _get_modules()
    f32, bf16, i32 = mybir.dt.float32, mybir.dt.bfloat16, mybir.dt.int32
    fp8 = mybir.dt.float8e4

    nc = bass.Bass("TRN2", target_bir_lowering=False, debug=False,
                   num_devices=NCORES)
    tissue_d = nc.dram_tensor("tissue", (CH, H, W), bf16, kind="ExternalInput").ap()
    cell_d = nc.dram_tensor("cell", (C, PHASES, PRR, PRC), fp8,
                            kind="ExternalInput").ap()
    w6t_d = nc.dram_tensor("w6t", (36, C, CH), fp8, kind="ExternalInput").ap()
    bias_d = nc.dram_tensor("bias", (CH, 1), f32, kind="ExternalInput").ap()
    roff_d = nc.dram_tensor("roff", (1, 2), i32, kind="ExternalInput").ap()
    out_d = nc.dram_tensor("out", (CH, H, W), bf16, kind="ExternalOutput").ap()

    NBLK = 8             # conv row blocks: 8 output rows, one PSUM bank each
    ORB = ROI // NBLK    # 8 output rows per block

    # taps grouped by polyphase plane so matmuls chase the plane DMAs
    tap_order = []       # (tap_idx, plane, row_shift, col_shift)
    for pp in range(4):
        for qq in range(4):
            for p in range(pp, 6, 4):
                for q in range(qq, 6, 4):
                    tap_order.append((p * 6 + q, pp * 4 + qq, p // 4, q // 4))
    assert len(tap_order) == 36

    with tile.TileContext(nc) as tc:
        with (
            tc.tile_pool(name="const", bufs=1) as constp,
            tc.tile_pool(name="cellp", bufs=1) as cellp,
            tc.tile_pool(name="roip", bufs=1) as roip,
            tc.tile_pool(name="psum", bufs=1, space="PSUM") as psump,
        ):
            # --- constants (scalar-engine DGE queue; tiny, first) ---
            roff_sb = constp.tile([1, 2], i32)
            nc.scalar.dma_start(roff_sb[:], roff_d[:])
            w_sb = constp.tile([C, 36 * CH], fp8)
            # w6t (36, C, CH) -> partition=input channel, free=(tap, out ch)
            nc.scalar.dma_start(w_sb[:], w6t_d.rearrange("t i o -> i t o"))
            bias_sb = constp.tile([CH, 1], f32)
            nc.scalar.dma_start(bias_sb[:], bias_d[:])

            # offsets are in-bounds by construction; the runtime assert's
            # ISA op miscompiles on this walrus build
            dyn_engines = (mybir.EngineType.SP, mybir.EngineType.Activation,
                           mybir.EngineType.Pool)
            r_v = nc.values_load(roff_sb[0:1, 0:1], engines=dyn_engines,
                                 min_val=0, max_val=H - ROI,
                                 skip_runtime_bounds_check=True)
            c_v = nc.values_load(roff_sb[0:1, 1:2], engines=dyn_engines,
                                 min_val=0, max_val=W - ROI,
                                 skip_runtime_bounds_check=True)

            # ROI source pixels: load early, overlaps with everything below
            roi_sb = roip.tile([CH, ROI * ROI], bf16)
            nc.gpsimd.dma_start(
                roi_sb[:], tissue_d[:, bass.ds(r_v, ROI), bass.ds(c_v, ROI)]
            )

            # --- polyphase cell planes, resident in SBUF. One DMA per
            # plane: the SDMA round-robins across queue lanes, so small
            # per-plane units complete early and the matmuls chase them
            # (grouped loads all complete late and stall the PE). ---
            cell_t = cellp.tile([C, PHASES * PRR * PRC], fp8)
            c4 = cell_t.rearrange("c (ph r w) -> c ph r w", r=PRR, w=PRC)
            for ph in range(PHASES):
                nc.sync.dma_start(c4[:, ph], cell_d[:, ph])

            # --- bulk copy tissue -> out: DRAM->DRAM, 128 KB contiguous
            # descriptors, no SBUF transit. Two channel-halves so the first
            # half of the dynamic ROI scatter can overlap the second half
            # of the copy. ---
            t_flat = tissue_d.rearrange("c h w -> c (h w)")
            o_flat = out_d.rearrange("c h w -> c (h w)")
            CHH = CH // 2
            nc.sync.dma_start(o_flat[:CHH], t_flat[:CHH])
            nc.sync.dma_start(o_flat[CHH:], t_flat[CHH:])

            # roi_sb += bias (frees the fused op below for the fp8 descale)
            nc.vector.tensor_scalar_add(roi_sb[:], roi_sb[:], bias_sb[:])

            # --- conv: 36 taps as 18 fp8 DoubleRow pairs x 8 blocks,
            # accumulating in 8 PSUM banks. Each DoubleRow pass contracts
            # two taps at once (2x PE throughput); the pair dim is an
            # overlapping custom-stride AP over the resident planes. ---
            import bass_rust

            def _pair_ap(base, delta):
                ap = base.copy()
                dims = ap.ap.to_list()
                ap.ap = bass_rust.VecI64Pair(
                    [dims[0], [delta, 2]] + dims[1:])
                return ap

            pairs = []   # (t1, t2, ph1, pb1, qb1, cell_delta)
            for k in range(0, 36, 2):
                t1, ph1, pb1, qb1 = tap_order[k]
                t2, ph2, pb2, qb2 = tap_order[k + 1]
                dc = ((ph2 - ph1) * PRR * PRC + (pb2 - pb1) * PRC
                      + (qb2 - qb1))
                assert dc > 0
                pairs.append((t1, t2, ph1, pb1, qb1, dc))

            pss = [psump.tile([CH, ORB * ROI], f32, name=f"bank{b}")
                   for b in range(NBLK)]
            for i, (t1, t2, ph, pb, qb, dc) in enumerate(pairs):
                dw = (t2 - t1) * CH
                for b in range(NBLK):
                    nc.tensor.matmul(
                        pss[b][:],
                        _pair_ap(w_sb[:, t1 * CH:(t1 + 1) * CH], dw),
                        _pair_ap(
                            c4[:, ph, b * ORB + pb:b * ORB + pb + ORB,
                               qb:qb + ROI], dc),
                        start=(i == 0),
                        stop=(i == 17),
                        perf_mode=mybir.MatmulPerfMode.DoubleRow,
                    )
            # roi strip b += psum[b]/WS  (fused on DVE)
            for b in range(NBLK):
                strip = slice(b * ORB * ROI, (b + 1) * ORB * ROI)
                nc.vector.scalar_tensor_tensor(
                    roi_sb[:, strip], pss[b][:], 1.0 / WS, roi_sb[:, strip],
                    mybir.AluOpType.mult, mybir.AluOpType.add,
                )

            # --- ROI scatter: overwrite after the bulk writes landed.
            # Instead of a full-engine barrier (expensive staggered sem
            # chains), each write engine stalls on the copy through a RAW
            # hazard: a 1-element read of the copied half. The scatter for
            # channels [0:32] starts as soon as copy half A lands,
            # overlapping copy half B; [32:64] follows half B. ---
            dummy = roip.tile([1, 8], bf16, name="wwsync")
            roi_dst = out_d[:, bass.ds(r_v, ROI), bass.ds(c_v, ROI)]
            splits = [(nc.sync, 0, 11, 32, 43, 0), (nc.scalar, 11, 21, 43, 53, 2),
                      (nc.gpsimd, 21, 32, 53, 64, 4)]
            for eng, a0, a1, b0, b1, i in splits:
                eng.dma_start(dummy[0:1, i:i + 1], o_flat[0:1, i:i + 1])
                eng.dma_start(roi_dst[a0:a1], roi_sb[a0:a1])
                eng.dma_start(dummy[0:1, i + 1:i + 2], o_flat[CHH:CHH + 1, i:i + 1])
                eng.dma_start(roi_dst[b0:b1], roi_sb[b0:b1])

    _split_multiwaits(nc, mybir)
    _CACHE["nc"] = nc
    return nc


def _prep_inputs(tissue_features, cell_features, loc, conv_w, conv_b):
    import ml_dtypes

    bf16 = ml_dtypes.bfloat16
    fp8 = ml_dtypes.float8_e4m3
    # fold AvgPool4x4 into the conv kernel: 6x6 taps
    w6 = np.zeros((C, C, 6, 6), np.float32)
    for dr in range(4):
        for dc in range(4):
            w6[:, :, dr:dr + 3, dc:dc + 3] += conv_w
    w6 *= WS / 16.0

    # polyphase split of the zero-padded cell map:
    # plane (pp,qq)[y,x] = padded[4y+pp, 4x+qq], padded = 1px zero border
    padc = np.zeros((C, 4 * PRC, 4 * PRC), np.float32)
    padc[:, 1:1 + H, 1:1 + W] = cell_features[0]
    cell_poly = np.empty((C, PHASES, PRR, PRC), np.float32)
    for pp in range(4):
        for qq in range(4):
            cell_poly[:, pp * 4 + qq] = padc[:, pp:pp + 4 * PRR:4, qq::4]
    cell_poly = np.ascontiguousarray(np.clip(cell_poly, -240, 240)).astype(fp8)

    w6t = {}
    bias = {}
    for h in range(2):
        sl = slice(CH * h, CH * (h + 1))
        # (CH, C, 6, 6) -> (tap, in ch, out ch)
        w6t[h] = np.ascontiguousarray(
            np.clip(w6[sl].transpose(2, 3, 1, 0).reshape(36, C, CH), -240, 240)
        ).astype(fp8)
        bias[h] = np.ascontiguousarray(conv_b[sl].astype(np.float32)).reshape(CH, 1)

    r0 = loc[:, 1].astype(np.int64) * W // 1024 - L   # H-dim start (from loc x)
    c0 = loc[:, 0].astype(np.int64) * W // 1024 - L   # W-dim start (from loc y)

    tissue_bf = tissue_features.astype(bf16)
    in_maps = []
    for c in range(NCORES):
        j, h = c % B, c // B
        in_maps.append({
            "tissue": tissue_bf[j, CH * h:CH * (h + 1)],
            "cell": cell_poly,
            "w6t": w6t[h],
            "bias": bias[h],
            "roff": np.array([[r0[j], c0[j]]], np.int32),
        })
    return in_maps


def run_device(tissue_features, cell_features, loc, conv_w, conv_b, **spmd_kwargs):
    """Build+run the SPMD kernel; returns (final (4,128,256,256), raw results)."""
    *_, run_bass_kernel_spmd = _get_modules()
    nc = _build_program()
    in_maps = _prep_inputs(tissue_features, cell_features, loc, conv_w, conv_b)
    res = run_bass_kernel_spmd(nc, in_maps, list(range(NCORES)), **spmd_kwargs)
    final = np.empty((B, C, H, W), np.float32)
    for c in range(NCORES):
        j, h = c % B, c // B
        final[j, CH * h:CH * (h + 1)] = res.results[c]["out"].astype(np.float32)
    return final, res


def kernel(tissue_features, cell_features, loc, conv_w, conv_b):
    final, _ = run_device(tissue_features, cell_features, loc, conv_w, conv_b)
    # reference stacks B copies of the fully-mutated tissue
    return np.broadcast_to(final[None], (B, B, C, H, W))


# revision 30
# speedup vs baseline: 1.5713x; 1.0004x over previous
"""Trainium2 Bass kernel for nn_Cell2Tissue (scatter_memory).

Reference computation:
  avg = AvgPool4x4(Conv3x3_SAME(cell) + bias)          # (128, 64, 64)
  for each tissue sample j: ROI_j += avg               # 64x64 ROI from loc
  output = stack of B copies of the mutated tissue     # (4, 4, 128, 256, 256)

Sharding over 8 cores: core c = (sample j = c % 4, channel half h = c // 4).

v2 strategy (memory-bound; tolerance gate is rel_err < 2e-2):
- tissue/out streamed as bf16 (host converts; ~0.23% rel err) -> bulk
  copy bytes halve to 16.8 MB/core.
- cell polyphase planes and folded 6x6 weights in fp8e4 (weights
  pre-scaled x2048, descaled in the DVE add) -> 8.8 MB/core replicated.
  Simulated end-to-end rel err 2.3e-3.
- bulk copy is a single DRAM->DRAM DMA (64 descriptors x 128 KB
  contiguous) instead of 32 strided through-SBUF tile DMAs.
- static-ring order: constants, cell planes, bulk copy - so matmuls
  chase the plane loads and finish under the copy.
- dynamic ROI read early on gpsimd; ROI write after an all-engine
  barrier, split across the 3 dynamic-capable queues (sync/scalar/
  gpsimd).
"""

import os
import numpy as np

B, C, H, W = 4, 128, 256, 256
CH = C // 2          # channels per core (half)
L = 32               # half ROI width
ROI = 2 * L          # 64
NCORES = 8
PRR = 65             # polyphase plane rows (max y+pb = 64)
PRC = 66             # polyphase plane cols
PHASES = 16
WS = 2048.0          # fp8 weight pre-scale (power of 2; descaled on DVE)

_CACHE = {}


def _get_modules():
    if "mods" in _CACHE:
        return _CACHE["mods"]
    # bass2jax executes via the jax 'axon'/'neuron' platform; a cpu-pinned
    # JAX_PLATFORMS would hide the devices.
    if os.environ.get("JAX_PLATFORMS") in ("cpu",):
        del os.environ["JAX_PLATFORMS"]
    import concourse.bass as bass
    import concourse.mybir as mybir
    import concourse.tile as tile
    from concourse.bass_utils import run_bass_kernel_spmd

    _CACHE["mods"] = (bass, mybir, tile, run_bass_kernel_spmd)
    return _CACHE["mods"]


def _split_multiwaits(nc, mybir, max_waits=1):
    """The walrus build here rejects >1 sem-wait on some instructions (the
    Tile tail InstDrain). Hoist extra waits onto single-wait nops placed
    immediately before, on the same engine (same-engine program order
    preserves semantics)."""
    for fn in nc.m.functions:
        for bb in fn.blocks:
            insts = bb.instructions
            i = 0
            while i < len(insts):
                inst = insts[i]
                si = inst.sync_info
                if si is not None and si.on_wait and len(si.on_wait) > max_waits:
                    waits = list(si.on_wait)
                    keep = waits[-max_waits:]
                    for k, w in enumerate(waits[:-max_waits]):
                        nop = mybir.InstNoOp(
                            name=f"{inst.name}_hoistwait_{k}",
                            sync_info=mybir.SyncInfo(on_wait=[w], on_update=[]),
                            bass_nofuse=True,
                            engine=inst.engine,
                        )
                        insts.insert(i, nop)
                        i += 1
                    si.on_wait = keep
                i += 1


def _build_program():
    """One SPMD program: per-core inputs
      tissue (64,256,256) bf16, cell (128,16,65,66) fp8e4 polyphase
      (replicated), w6t (36,128,64) fp8e4 x2048, bias (64,1) f32,
      roff (1,2) i32 = [row0, col0]
    output: out (64,256,256) bf16 = tissue with avg added in the ROI."""
    if "nc" in _CACHE:
        return _CACHE["nc"]
    bass, mybir, tile, _ = _get_modules()
    f32, bf16, i32 = mybir.dt.float32, mybir.dt.bfloat16, mybir.dt.int32
    fp8 = mybir.dt.float8e4

    nc = bass.Bass("TRN2", target_bir_lowering=False, debug=False,
                   num_devices=NCORES)
    tissue_d = nc.dram_tensor("tissue", (CH, H, W), bf16, kind="ExternalInput").ap()
    cell_d = nc.dram_tensor("cell", (C, PHASES, PRR, PRC), fp8,
                            kind="ExternalInput").ap()
    w6t_d = nc.dram_tensor("w6t", (36, C, CH), fp8, kind="ExternalInput").ap()
    bias_d = nc.dram_tensor("bias", (CH, 1), f32, kind="ExternalInput").ap()
    roff_d = nc.dram_tensor("roff", (1, 2), i32, kind="ExternalInput").ap()
    out_d = nc.dram_tensor("out", (CH, H, W), bf16, kind="ExternalOutput").ap()

    NBLK = 8             # conv row blocks: 8 output rows, one PSUM bank each
    ORB = ROI // NBLK    # 8 output rows per block

    # taps grouped by polyphase plane so matmuls chase the plane DMAs
    tap_order = []       # (tap_idx, plane, row_shift, col_shift)
    for pp in range(4):
        for qq in range(4):
            for p in range(pp, 6, 4):
                for q in range(qq, 6, 4):
                    tap_order.append((p * 6 + q, pp * 4 + qq, p // 4, q // 4))
    assert len(tap_order) == 36

    with tile.TileContext(nc) as tc:
        with (
            tc.tile_pool(name="const", bufs=1) as constp,
            tc.tile_pool(name="cellp", bufs=1) as cellp,
            tc.tile_pool(name="roip", bufs=1) as roip,
            tc.tile_pool(name="psum", bufs=1, space="PSUM") as psump,
        ):
            # --- constants (scalar-engine DGE queue; tiny, first) ---
            roff_sb = constp.tile([1, 2], i32)
            nc.scalar.dma_start(roff_sb[:], roff_d[:])
            w_sb = constp.tile([C, 36 * CH], fp8)
            # w6t (36, C, CH) -> partition=input channel, free=(tap, out ch)
            nc.scalar.dma_start(w_sb[:], w6t_d.rearrange("t i o -> i t o"))
            bias_sb = constp.tile([CH, 1], f32)
            nc.scalar.dma_start(bias_sb[:], bias_d[:])

            # offsets are in-bounds by construction; the runtime assert's
            # ISA op miscompiles on this walrus build
            dyn_engines = (mybir.EngineType.SP, mybir.EngineType.Activation,
                           mybir.EngineType.Pool)
            r_v = nc.values_load(roff_sb[0:1, 0:1], engines=dyn_engines,
                                 min_val=0, max_val=H - ROI,
                                 skip_runtime_bounds_check=True)
            c_v = nc.values_load(roff_sb[0:1, 1:2], engines=dyn_engines,
                                 min_val=0, max_val=W - ROI,
                                 skip_runtime_bounds_check=True)

            # ROI source pixels: load early, overlaps with everything below
            roi_sb = roip.tile([CH, ROI * ROI], bf16)
            nc.gpsimd.dma_start(
                roi_sb[:], tissue_d[:, bass.ds(r_v, ROI), bass.ds(c_v, ROI)]
            )

            # --- polyphase cell planes, resident in SBUF. One DMA per
            # plane: the SDMA round-robins across queue lanes, so small
            # per-plane units complete early and the matmuls chase them
            # (grouped loads all complete late and stall the PE). ---
            cell_t = cellp.tile([C, PHASES * PRR * PRC], fp8)
            c4 = cell_t.rearrange("c (ph r w) -> c ph r w", r=PRR, w=PRC)
            for ph in range(PHASES):
                nc.sync.dma_start(c4[:, ph], cell_d[:, ph])

            # --- bulk copy tissue -> out: DRAM->DRAM, 128 KB contiguous
            # descriptors, no SBUF transit. Two channel-halves so the first
            # half of the dynamic ROI scatter can overlap the second half
            # of the copy. ---
            t_flat = tissue_d.rearrange("c h w -> c (h w)")
            o_flat = out_d.rearrange("c h w -> c (h w)")
            CHH = CH // 2
            nc.sync.dma_start(o_flat[:CHH], t_flat[:CHH])
            nc.sync.dma_start(o_flat[CHH:], t_flat[CHH:])

            # roi_sb += bias (frees the fused op below for the fp8 descale)
            nc.vector.tensor_scalar_add(roi_sb[:], roi_sb[:], bias_sb[:])

            # --- conv: 36 taps as 18 fp8 DoubleRow pairs x 8 blocks,
            # accumulating in 8 PSUM banks. Each DoubleRow pass contracts
            # two taps at once (2x PE throughput); the pair dim is an
            # overlapping custom-stride AP over the resident planes. ---
            import bass_rust

            def _pair_ap(base, delta):
                ap = base.copy()
                dims = ap.ap.to_list()
                ap.ap = bass_rust.VecI64Pair(
                    [dims[0], [delta, 2]] + dims[1:])
                return ap

            pairs = []   # (t1, t2, ph1, pb1, qb1, cell_delta)
            for k in range(0, 36, 2):
                t1, ph1, pb1, qb1 = tap_order[k]
                t2, ph2, pb2, qb2 = tap_order[k + 1]
                dc = ((ph2 - ph1) * PRR * PRC + (pb2 - pb1) * PRC
                      + (qb2 - qb1))
                assert dc > 0
                pairs.append((t1, t2, ph1, pb1, qb1, dc))

            pss = [psump.tile([CH, ORB * ROI], f32, name=f"bank{b}")
                   for b in range(NBLK)]
            for i, (t1, t2, ph, pb, qb, dc) in enumerate(pairs):
                dw = (t2 - t1) * CH
                for b in range(NBLK):
                    nc.tensor.matmul(
                        pss[b][:],
                        _pair_ap(w_sb[:, t1 * CH:(t1 + 1) * CH], dw),
                        _pair_ap(
                            c4[:, ph, b * ORB + pb:b * ORB + pb + ORB,
                               qb:qb + ROI], dc),
                        start=(i == 0),
                        stop=(i == 17),
                        perf_mode=mybir.MatmulPerfMode.DoubleRow,
                    )
            # roi strip b += psum[b]/WS  (fused on DVE)
            for b in range(NBLK):
                strip = slice(b * ORB * ROI, (b + 1) * ORB * ROI)
                nc.vector.scalar_tensor_tensor(
                    roi_sb[:, strip], pss[b][:], 1.0 / WS, roi_sb[:, strip],
                    mybir.AluOpType.mult, mybir.AluOpType.add,
                )

            # --- ROI scatter: overwrite after the bulk writes landed.
            # Instead of a full-engine barrier (expensive staggered sem
            # chains), each write engine stalls on the copy through a RAW
            # hazard: a 1-element read of the copied half. The scatter for
            # channels [0:32] starts as soon as copy half A lands,
            # overlapping copy half B; [32:64] follows half B. ---
            dummy = roip.tile([1, 8], bf16, name="wwsync")
            roi_dst = out_d[:, bass.ds(r_v, ROI), bass.ds(c_v, ROI)]
            splits = [(nc.scalar, 0, 32, 32, 64, 0)]
            for eng, a0, a1, b0, b1, i in splits:
                eng.dma_start(dummy[0:1, i:i + 1], o_flat[0:1, i:i + 1])
                eng.dma_start(roi_dst[a0:a1], roi_sb[a0:a1])
                eng.dma_start(dummy[0:1, i + 1:i + 2], o_flat[CHH:CHH + 1, i:i + 1])
                eng.dma_start(roi_dst[b0:b1], roi_sb[b0:b1])

    _split_multiwaits(nc, mybir)
    _CACHE["nc"] = nc
    return nc


def _prep_inputs(tissue_features, cell_features, loc, conv_w, conv_b):
    import ml_dtypes

    bf16 = ml_dtypes.bfloat16
    fp8 = ml_dtypes.float8_e4m3
    # fold AvgPool4x4 into the conv kernel: 6x6 taps
    w6 = np.zeros((C, C, 6, 6), np.float32)
    for dr in range(4):
        for dc in range(4):
            w6[:, :, dr:dr + 3, dc:dc + 3] += conv_w
    w6 *= WS / 16.0

    # polyphase split of the zero-padded cell map:
    # plane (pp,qq)[y,x] = padded[4y+pp, 4x+qq], padded = 1px zero border
    padc = np.zeros((C, 4 * PRC, 4 * PRC), np.float32)
    padc[:, 1:1 + H, 1:1 + W] = cell_features[0]
    cell_poly = np.empty((C, PHASES, PRR, PRC), np.float32)
    for pp in range(4):
        for qq in range(4):
            cell_poly[:, pp * 4 + qq] = padc[:, pp:pp + 4 * PRR:4, qq::4]
    cell_poly = np.ascontiguousarray(np.clip(cell_poly, -240, 240)).astype(fp8)

    w6t = {}
    bias = {}
    for h in range(2):
        sl = slice(CH * h, CH * (h + 1))
        # (CH, C, 6, 6) -> (tap, in ch, out ch)
        w6t[h] = np.ascontiguousarray(
            np.clip(w6[sl].transpose(2, 3, 1, 0).reshape(36, C, CH), -240, 240)
        ).astype(fp8)
        bias[h] = np.ascontiguousarray(conv_b[sl].astype(np.float32)).reshape(CH, 1)

    r0 = loc[:, 1].astype(np.int64) * W // 1024 - L   # H-dim start (from loc x)
    c0 = loc[:, 0].astype(np.int64) * W // 1024 - L   # W-dim start (from loc y)

    tissue_bf = tissue_features.astype(bf16)
    in_maps = []
    for c in range(NCORES):
        j, h = c % B, c // B
        in_maps.append({
            "tissue": tissue_bf[j, CH * h:CH * (h + 1)],
            "cell": cell_poly,
            "w6t": w6t[h],
            "bias": bias[h],
            "roff": np.array([[r0[j], c0[j]]], np.int32),
        })
    return in_maps


def run_device(tissue_features, cell_features, loc, conv_w, conv_b, **spmd_kwargs):
    """Build+run the SPMD kernel; returns (final (4,128,256,256), raw results)."""
    *_, run_bass_kernel_spmd = _get_modules()
    nc = _build_program()
    in_maps = _prep_inputs(tissue_features, cell_features, loc, conv_w, conv_b)
    res = run_bass_kernel_spmd(nc, in_maps, list(range(NCORES)), **spmd_kwargs)
    final = np.empty((B, C, H, W), np.float32)
    for c in range(NCORES):
        j, h = c % B, c // B
        final[j, CH * h:CH * (h + 1)] = res.results[c]["out"].astype(np.float32)
    return final, res


def kernel(tissue_features, cell_features, loc, conv_w, conv_b):
    final, _ = run_device(tissue_features, cell_features, loc, conv_w, conv_b)
    # reference stacks B copies of the fully-mutated tissue
    return np.broadcast_to(final[None], (B, B, C, H, W))
